# revision 2
# baseline (speedup 1.0000x reference)
"""Trainium2 Bass kernel v2 for GQA attention (B=2,T=2048,D=2048,H=16,G=4,K=128)
with QK RMS-norm, RoPE, segment-aware causal masking, sigmoid gating, o_proj.

Sharding: 8 cores = (batch b, kv-group g); core c -> b=c//4, g=c%4.
v2 changes vs baseline:
  - all transposes on the PE (is_transpose matmul, ~73ns) instead of DMA
    transposes (~1.3us each, serialized on the sync queue)
  - logits batched over 4 heads per (i,j) tile: exact tile-granular schedule
    at full 512-col matmul efficiency (27k cols vs 47k chunk-union)
  - gate projection in fp8 (e4m3) DoubleRow at 2x matmul rate
  - phase-interleaved emission: logits/PV/o_proj of chunk c zip with
    projections of chunk c+1 so exp/evictions hide under matmuls
  - host-prepacked weight/table layouts (contiguous SBUF-shaped DMAs)
"""
import sys

sys.path.insert(0, "/opt/trn_rl_repo")

import numpy as np
import ml_dtypes

import concourse.bass as bass
import concourse.mybir as mybir
import concourse.tile as tile
from concourse.bass_utils import run_bass_kernel_spmd
from concourse.masks import make_identity

FP32 = mybir.dt.float32
BF16 = mybir.dt.bfloat16
F8 = mybir.dt.float8e4
AF = mybir.ActivationFunctionType
ALU = mybir.AluOpType
BF = ml_dtypes.bfloat16
F8NP = ml_dtypes.float8_e4m3

B, T, D = 2, 2048, 2048
H, G, K = 16, 4, 128
HPC = H // G              # q-heads per core = 4
EPS = 1e-6
SCALE = K ** -0.5
NT = T // 128             # 16 t-tiles
NDC = D // 128            # 16 d-chunks
EMPTY, FULL, PARTIAL = 0, 1, 2
GATE_F8 = True
GSC = 32.0                # gate weight pre-scale (fp8 normal range)
P = 128


def split_multiwaits(nc):
    """This container's walrus accepts one sync-wait per instruction; hoist
    extras into standalone single-wait EventSemaphore instructions."""
    n = 0
    for fn in nc.m.functions:
        for bb in fn.blocks:
            out = []
            for ins in bb.instructions:
                si = ins.sync_info
                if si is not None and si.on_wait and len(si.on_wait) > 1:
                    waits = list(si.on_wait)
                    for w in waits[:-1]:
                        n += 1
                        out.append(mybir.InstEventSemaphore(
                            name=f"{ins.name}-w{n}", engine=ins.engine,
                            ins=[], outs=[],
                            sync_info=mybir.SyncInfo(on_wait=[w], on_update=[])))
                    ins.sync_info = mybir.SyncInfo(
                        on_wait=[waits[-1]], on_update=list(si.on_update or []))
                out.append(ins)
            bb.instructions = out
    return n


def _classify(allowed):
    """allowed: [T,T] bool (t,s). Returns cls[NT,NT] in {EMPTY,FULL,PARTIAL}."""
    cls = np.zeros((NT, NT), np.int32)
    a4 = allowed.reshape(NT, 128, NT, 128)
    any_ = a4.any(axis=(1, 3))
    all_ = a4.all(axis=(1, 3))
    cls[any_ & all_] = FULL
    cls[any_ & ~all_] = PARTIAL
    return cls


def _build_schedule(segment_ids, position_ids):
    """Union schedule across batches (SPMD: one program for all cores) plus
    per-batch mask tiles for partial (i,j)."""
    allowed = []
    for b in range(B):
        pos = position_ids[b].astype(np.int64)
        seg = segment_ids[b].astype(np.int64)
        al = (pos[:, None] >= pos[None, :]) & (seg[:, None] == seg[None, :])
        allowed.append(al)
    cls_b = [_classify(al) for al in allowed]
    cls = np.maximum(cls_b[0], cls_b[1])
    cls[(cls_b[0] == FULL) & (cls_b[1] == PARTIAL)] = PARTIAL
    cls[(cls_b[1] == FULL) & (cls_b[0] == PARTIAL)] = PARTIAL

    mask_idx = {}
    masksT = [[], []]   # per batch: list of [128s,128t] f32 (deduped pairs)
    seen = {}
    for i in range(NT):
        for j in range(NT):
            if cls[i, j] == PARTIAL:
                subs = [allowed[b][i * 128:(i + 1) * 128, j * 128:(j + 1) * 128].T
                        for b in range(B)]
                key = subs[0].tobytes() + subs[1].tobytes()
                if key not in seen:
                    seen[key] = len(masksT[0])
                    for b in range(B):
                        masksT[b].append(subs[b].astype(np.float32))
                mask_idx[(i, j)] = seen[key]
    pv = {i: [j for j in range(NT) if cls[i, j] != EMPTY] for i in range(NT)}
    return cls, mask_idx, masksT, pv


def _build_program(n_masks, cls, mask_idx, pv):
    nc = bass.Bass()
    hid = nc.declare_dram_parameter("hid", [P, NDC * T], BF16, isOutput=False)
    wq = nc.declare_dram_parameter("wq", [P, NDC * 512], BF16, isOutput=False)
    wkv = nc.declare_dram_parameter("wkv", [P, NDC * 256], BF16, isOutput=False)
    if GATE_F8:
        hid8 = nc.declare_dram_parameter("hid8", [P, NDC * T], F8, isOutput=False)
        wg = nc.declare_dram_parameter("wg", [P, 8 * 1024], F8, isOutput=False)
    else:
        wg = nc.declare_dram_parameter("wg", [P, NDC * 512], BF16, isOutput=False)
    wo = nc.declare_dram_parameter("wo", [P, HPC * D], BF16, isOutput=False)
    tabs_d = {}
    for nm in ("cq", "sq", "ck", "sk"):
        tabs_d[nm] = nc.declare_dram_parameter(nm, [P, NT * K], BF16, isOutput=False)
    masks = nc.declare_dram_parameter("masks", [P, max(n_masks, 1) * 128], BF16,
                                      isOutput=False)
    out = nc.declare_dram_parameter("out", [T, D], BF16, isOutput=True)

    with tile.TileContext(nc) as tc:
        with tc.tile_pool(name="res", bufs=1) as res, \
             tc.tile_pool(name="hidp", bufs=4) as hidp, \
             tc.tile_pool(name="hid8p", bufs=4) as hid8p, \
             tc.tile_pool(name="ps_q", bufs=2, space="PSUM") as ps_q, \
             tc.tile_pool(name="ps_sm", bufs=4, space="PSUM") as ps_sm, \
             tc.tile_pool(name="ps_lg", bufs=2, space="PSUM") as ps_lg, \
             tc.tile_pool(name="ptp", bufs=16) as ptp, \
             tc.tile_pool(name="stage", bufs=3) as stage:

            # ---- resident loads, ordered so the PE can start immediately ----
            wq_sb = res.tile([P, NDC * 512], BF16, tag="wq")
            wkv_sb = res.tile([P, NDC * 256], BF16, tag="wkv")
            nc.sync.dma_start(out=wq_sb[:, 0:1024], in_=wq[:, 0:1024])
            nc.sync.dma_start(out=wkv_sb[:, 0:512], in_=wkv[:, 0:512])
            nc.sync.dma_start(out=wq_sb[:, 1024:2048], in_=wq[:, 1024:2048])
            nc.sync.dma_start(out=wkv_sb[:, 512:1024], in_=wkv[:, 512:1024])
            nc.sync.dma_start(out=wq_sb[:, 2048:4096], in_=wq[:, 2048:4096])
            nc.sync.dma_start(out=wkv_sb[:, 1024:2048], in_=wkv[:, 1024:2048])

            ht_tiles = {}

            def get_ht(pr, nsplit=1, f8_first=False):
                if pr in ht_tiles or pr >= NT // 2:
                    return

                def _ld8():
                    if GATE_F8:
                        h8 = hid8p.tile([P, NDC * 256], F8, tag="hid8T",
                                        name=f"h8_{pr}")
                        nc.gpsimd.dma_start(
                            out=h8[:].rearrange("p (c j) -> p c j", c=NDC),
                            in_=hid8[:].rearrange("p (c t) -> p c t", c=NDC)[
                                :, :, pr * 256:(pr + 1) * 256])
                        ht_tiles[(pr, 8)] = h8

                if f8_first:
                    _ld8()
                ht = hidp.tile([P, NDC * 256], BF16, tag="hidT", name=f"ht_{pr}")
                dpc = NDC // nsplit
                for spl in range(nsplit):
                    c0, c1 = spl * dpc, (spl + 1) * dpc
                    nc.gpsimd.dma_start(
                        out=ht[:].rearrange("p (c j) -> p c j", c=NDC)[:, c0:c1],
                        in_=hid[:].rearrange("p (c t) -> p c t", c=NDC)[
                            :, c0:c1, pr * 256:(pr + 1) * 256])
                ht_tiles[pr] = ht
                if not f8_first:
                    _ld8()

            get_ht(0, nsplit=4, f8_first=True)
            for ch in range(2, 4):
                nc.sync.dma_start(out=wq_sb[:, ch * 2048:(ch + 1) * 2048],
                                  in_=wq[:, ch * 2048:(ch + 1) * 2048])
                nc.sync.dma_start(out=wkv_sb[:, ch * 1024:(ch + 1) * 1024],
                                  in_=wkv[:, ch * 1024:(ch + 1) * 1024])
            tabs = {}
            for nm in ("cq", "sq", "ck", "sk"):
                tt_ = res.tile([P, NT * K], BF16, tag=f"tab{nm}")
                nc.sync.dma_start(out=tt_[:], in_=tabs_d[nm][:])
                tabs[nm] = tt_
            if GATE_F8:
                wg_sb = res.tile([P, 8 * 1024], F8, tag="wg")
            else:
                wg_sb = res.tile([P, NDC * 512], BF16, tag="wg")
            nc.gpsimd.dma_start(out=wg_sb[:], in_=wg[:])
            get_ht(1, f8_first=True)
            get_ht(2)
            get_ht(3, f8_first=True)
            mask_sb = res.tile([P, max(n_masks, 1) * 128], BF16, tag="masks")
            nc.gpsimd.dma_start(out=mask_sb[:], in_=masks[:])
            wo_sb = res.tile([P, HPC * D], BF16, tag="wo")
            nc.gpsimd.dma_start(out=wo_sb[:], in_=wo[:])

            # persistent tensors
            qT = res.tile([P, HPC * T], BF16, tag="qT")          # [k, h, t]
            kT = res.tile([P, T], BF16, tag="kT")                # [k, t]
            v_sb = res.tile([P, NT * 130], BF16, tag="v")        # [s, j*130+k], col128=1
            sg = res.tile([P, NT * 512], BF16, tag="sg")         # [t, i, h*128+k]
            epsb = res.tile([P, 1], FP32, tag="eps")
            ident = res.tile([P, P], BF16, tag="ident")
            make_identity(nc, ident)
            nc.vector.memset(epsb[:], EPS)
            nc.vector.memset(v_sb[:], 1.0)

            qT3 = qT[:].rearrange("p (h t) -> p h t", h=HPC)
            pending_raw = {}
            pending_ssb = {}

            def _proj_mms(tt, qps, kvps):
                pr = tt // 2
                ht = ht_tiles[pr]
                off = (tt % 2) * 128
                for dc in range(NDC):
                    lhsT = ht[:, dc * 256 + off: dc * 256 + off + 128]
                    st, sp = dc == 0, dc == NDC - 1
                    nc.tensor.matmul(qps[:], lhsT, wq_sb[:, dc * 512:(dc + 1) * 512],
                                     start=st, stop=sp)
                    nc.tensor.matmul(kvps[:], lhsT, wkv_sb[:, dc * 256:(dc + 1) * 256],
                                     start=st, stop=sp)

            def _gate_mms(tt, pool=None, tag="sm"):
                pr = tt // 2
                off = (tt % 2) * 128
                gps = (pool or ps_sm).tile([P, 512], FP32, tag=tag, name=f"gps{tt}")
                if GATE_F8:
                    h8 = ht_tiles[(pr, 8)]
                    h83 = h8[:].rearrange("p (c j) -> p c j", c=NDC)
                    wg3 = wg_sb[:].rearrange("p (a two f) -> p a two f", a=8, two=2)
                    for a in range(8):
                        nc.tensor.matmul(gps[:], h83[:, 2 * a:2 * a + 2, off:off + 128],
                                         wg3[:, a], start=(a == 0), stop=(a == 7),
                                         perf_mode=mybir.MatmulPerfMode.DoubleRow)
                else:
                    ht = ht_tiles[pr]
                    for dc in range(NDC):
                        lhsT = ht[:, dc * 256 + off: dc * 256 + off + 128]
                        nc.tensor.matmul(gps[:], lhsT, wg_sb[:, dc * 512:(dc + 1) * 512],
                                         start=(dc == 0), stop=(dc == NDC - 1))
                return gps

            def _qkv_evict(tt, qps, kvps, ssb):
                ic = tt % 4
                scr = stage.tile([P, 128], FP32, tag="scr")
                for h in range(HPC):
                    nc.scalar.activation(scr[:], qps[:, h * 128:(h + 1) * 128],
                                         AF.Square,
                                         accum_out=ssb[:, ic * 5 + h: ic * 5 + h + 1])
                nc.scalar.activation(scr[:], kvps[:, 0:128], AF.Square,
                                     accum_out=ssb[:, ic * 5 + 4: ic * 5 + 5])
                qraw = stage.tile([P, 512], BF16, tag="qraw", bufs=5)
                kraw = stage.tile([P, 128], BF16, tag="kraw", bufs=6)
                nc.vector.tensor_copy(qraw[:], qps[:])
                nc.vector.tensor_copy(kraw[:], kvps[:, 0:128])
                nc.vector.tensor_copy(v_sb[:, tt * 130:tt * 130 + 128],
                                      kvps[:, 128:256])
                pending_raw[tt] = (qraw, kraw)

            def _gate_evict(tt, gps):
                sgc = sg[:, tt * 512:(tt + 1) * 512]
                nc.scalar.activation(sgc, gps[:], AF.Tanh,
                                     scale=(0.5 / GSC) if GATE_F8 else 0.5)
                nc.vector.tensor_scalar(sgc, sgc, 0.5, 0.5, ALU.mult, ALU.add)

            def _proj_evict(tt, qps, kvps, gps, ssb):
                _qkv_evict(tt, qps, kvps, ssb)
                _gate_evict(tt, gps)

            def emit_proj(tt, ssb):
                if tt % 2 == 0:
                    get_ht(tt // 2 + 1)
                qps = ps_q.tile([P, 512], FP32, tag="q")
                kvps = ps_sm.tile([P, 256], FP32, tag="sm")
                _proj_mms(tt, qps, kvps)
                gps = _gate_mms(tt)
                _proj_evict(tt, qps, kvps, gps, ssb)

            def emit_proj_pair(tta, ttb, ssb):
                """Prologue pair: fp8 gates first (only need ht8+wg, ~1.5MB),
                then the dc-interleaved qkv pair consumes each wq chunk as it
                lands. Gates use the idle lg PSUM slots."""
                get_ht(tta // 2 + 1)
                gA = _gate_mms(tta, pool=ps_lg, tag="lg")
                _gate_evict(tta, gA)
                gB = _gate_mms(ttb, pool=ps_lg, tag="lg")
                _gate_evict(ttb, gB)
                qpsA = ps_q.tile([P, 512], FP32, tag="q")
                kvA = ps_sm.tile([P, 256], FP32, tag="sm")
                qpsB = ps_q.tile([P, 512], FP32, tag="q")
                kvB = ps_sm.tile([P, 256], FP32, tag="sm")
                prA, offA = tta // 2, (tta % 2) * 128
                prB, offB = ttb // 2, (ttb % 2) * 128
                for dc in range(NDC):
                    st, sp = dc == 0, dc == NDC - 1
                    lA = ht_tiles[prA][:, dc * 256 + offA: dc * 256 + offA + 128]
                    lB = ht_tiles[prB][:, dc * 256 + offB: dc * 256 + offB + 128]
                    nc.tensor.matmul(qpsA[:], lA, wq_sb[:, dc * 512:(dc + 1) * 512],
                                     start=st, stop=sp)
                    nc.tensor.matmul(qpsB[:], lB, wq_sb[:, dc * 512:(dc + 1) * 512],
                                     start=st, stop=sp)
                    nc.tensor.matmul(kvA[:], lA, wkv_sb[:, dc * 256:(dc + 1) * 256],
                                     start=st, stop=sp)
                    nc.tensor.matmul(kvB[:], lB, wkv_sb[:, dc * 256:(dc + 1) * 256],
                                     start=st, stop=sp)
                _qkv_evict(tta, qpsA, kvA, ssb)
                _qkv_evict(ttb, qpsB, kvB, ssb)

            def emit_proj_group(tts, paired=False):
                ssb = stage.tile([P, 20], FP32, tag="ssb", bufs=2)
                if paired:
                    emit_proj_pair(tts[0], tts[1], ssb)
                    emit_proj_pair(tts[2], tts[3], ssb)
                else:
                    for tt in tts:
                        emit_proj(tt, ssb)
                pending_ssb[tts[0] // 4] = ssb

            def emit_nrt(i, irb):
                """RoPE on raw q/k, apply inv-rms, PE-transpose into qT/kT."""
                ic = i % 4
                qraw, kraw = pending_raw.pop(i)
                qr = stage.tile([P, 512], BF16, tag="qr", bufs=3)
                tmp = stage.tile([P, 512], BF16, tag="tmp")
                cqt = tabs["cq"][:, i * K:(i + 1) * K]
                sqt = tabs["sq"][:, i * K:(i + 1) * K]
                nc.vector.tensor_tensor(
                    tmp[:].rearrange("p (a k) -> p a k", k=128),
                    qraw[:].rearrange("p (a k) -> p a k", k=128),
                    cqt[:, None, :].to_broadcast((P, HPC, K)), ALU.mult)
                nc.vector.tensor_tensor(
                    qr[:].rearrange("p (a k) -> p a k", k=128)[:, :, 0:64],
                    qraw[:].rearrange("p (a k) -> p a k", k=128)[:, :, 64:128],
                    sqt[:, None, 0:64].to_broadcast((P, HPC, 64)), ALU.mult)
                nc.vector.tensor_tensor(
                    qr[:].rearrange("p (a k) -> p a k", k=128)[:, :, 64:128],
                    qraw[:].rearrange("p (a k) -> p a k", k=128)[:, :, 0:64],
                    sqt[:, None, 64:128].to_broadcast((P, HPC, 64)), ALU.mult)
                nc.vector.tensor_tensor(qr[:], qr[:], tmp[:], ALU.add)
                for h in range(HPC):
                    nc.vector.tensor_scalar_mul(
                        qr[:, h * 128:(h + 1) * 128], qr[:, h * 128:(h + 1) * 128],
                        irb[:, ic * 5 + h: ic * 5 + h + 1])
                kr = stage.tile([P, 128], BF16, tag="kr", bufs=3)
                tmpk = stage.tile([P, 128], BF16, tag="tmpk")
                ckt = tabs["ck"][:, i * K:(i + 1) * K]
                skt = tabs["sk"][:, i * K:(i + 1) * K]
                nc.vector.tensor_tensor(tmpk[:], kraw[:], ckt, ALU.mult)
                nc.vector.tensor_tensor(kr[:, 0:64], kraw[:, 64:128], skt[:, 0:64],
                                        ALU.mult)
                nc.vector.tensor_tensor(kr[:, 64:128], kraw[:, 0:64], skt[:, 64:128],
                                        ALU.mult)
                nc.vector.tensor_tensor(kr[:], kr[:], tmpk[:], ALU.add)
                nc.vector.tensor_scalar_mul(kr[:], kr[:],
                                            irb[:, ic * 5 + 4: ic * 5 + 5])
                for h in range(HPC):
                    tp = ps_sm.tile([P, 128], BF16, tag="sm")
                    nc.tensor.transpose(tp[:], qr[:, h * 128:(h + 1) * 128], ident[:])
                    nc.vector.tensor_copy(
                        qT[:, h * T + i * 128: h * T + (i + 1) * 128], tp[:])
                tp = ps_sm.tile([P, 128], BF16, tag="sm")
                nc.tensor.transpose(tp[:], kr[:], ident[:])
                nc.vector.tensor_copy(kT[:, i * 128:(i + 1) * 128], tp[:])

            def emit_nrt_chunk(c):
                irb = stage.tile([P, 20], FP32, tag="irb", bufs=2)
                ssb = pending_ssb.pop(c)
                nc.vector.tensor_scalar(irb[:], ssb[:], 1.0 / K, EPS,
                                        ALU.mult, ALU.add)
                nc.vector.reciprocal(irb[:], irb[:])
                nc.scalar.activation(irb[:], irb[:], AF.Sqrt)
                for r in range(4):
                    emit_nrt(c * 4 + r, irb)

            def emit_logits_chunk(c, pt_of, rs=range(4)):
                for r in rs:
                    i = c * 4 + r
                    for j in pv[i]:
                        lg = ps_lg.tile([P, 512], FP32, tag="lg")
                        nc.tensor.matmul(lg[:], kT[:, j * 128:(j + 1) * 128],
                                         qT3[:, :, i * 128:(i + 1) * 128],
                                         start=True, stop=True)
                        pt = ptp.tile([P, 512], BF16, tag="pt")
                        nc.scalar.activation(pt[:], lg[:], AF.Exp)
                        if cls[i, j] == PARTIAL:
                            m = mask_idx[(i, j)]
                            pt3 = pt[:].rearrange("p (h t) -> p h t", h=HPC)
                            nc.gpsimd.tensor_tensor(
                                pt3, pt3,
                                mask_sb[:, m * 128:(m + 1) * 128][:, None, :]
                                .to_broadcast((P, HPC, 128)), ALU.mult)
                        pt_of[(i, j)] = pt

            def gen_pv_i(c, r, pt_of, agc):
                i = c * 4 + r
                jl = pv[i]
                for h in range(HPC):
                    pvp = ps_sm.tile([P, 512], FP32, tag="sm", name=f"pvp{i}_{h}")
                    for nj, j in enumerate(jl):
                        nc.tensor.matmul(
                            pvp[:, 0:129],
                            pt_of[(i, j)][:, h * 128:(h + 1) * 128],
                            v_sb[:, j * 130:j * 130 + 129],
                            start=(nj == 0), stop=(nj == len(jl) - 1))
                        yield
                    rd = stage.tile([P, 1], FP32, tag="rd", bufs=4,
                                    name=f"rd{i}_{h}")
                    nc.vector.reciprocal(rd[:], pvp[:, 128:129])
                    nc.vector.scalar_tensor_tensor(
                        agc[:, r * 512 + h * 128: r * 512 + (h + 1) * 128],
                        pvp[:, 0:128], rd[:],
                        sg[:, i * 512 + h * 128: i * 512 + (h + 1) * 128],
                        ALU.mult, ALU.mult)

            def gen_oproj_i(c, r, agc):
                i = c * 4 + r
                agT = stage.tile([P, 512], BF16, tag="agT", bufs=4,
                                 name=f"agT{i}")
                for h in range(HPC):
                    tp = ps_sm.tile([P, 128], BF16, tag="sm", name=f"atp{i}_{h}")
                    nc.tensor.transpose(
                        tp[:], agc[:, r * 512 + h * 128: r * 512 + (h + 1) * 128],
                        ident[:])
                    yield
                    nc.scalar.activation(agT[:, h * 128:(h + 1) * 128], tp[:],
                                         AF.Copy)
                oo = stage.tile([P, D], BF16, tag="oo", bufs=2, name=f"oo{i}")
                for dc4 in range(4):
                    ops = ps_lg.tile([P, 512], FP32, tag="lg", name=f"ops{i}_{dc4}")
                    for h in range(HPC):
                        nc.tensor.matmul(
                            ops[:], agT[:, h * 128:(h + 1) * 128],
                            wo_sb[:, h * D + dc4 * 512: h * D + (dc4 + 1) * 512],
                            start=(h == 0), stop=(h == HPC - 1))
                        yield
                    if dc4 % 2 == 0:
                        nc.vector.tensor_copy(oo[:, dc4 * 512:(dc4 + 1) * 512],
                                              ops[:])
                    else:
                        nc.scalar.activation(oo[:, dc4 * 512:(dc4 + 1) * 512],
                                             ops[:], AF.Copy)
                nc.gpsimd.dma_start(out=out[i * 128:(i + 1) * 128, :], in_=oo[:])

            def _drive(*gens):
                gens = [g for g in gens if g is not None]
                while gens:
                    nxt = []
                    for g in gens:
                        try:
                            next(g)
                            nxt.append(g)
                        except StopIteration:
                            pass
                    gens = nxt

            def emit_pv_houter(c, pt_of, agc, rs):
                for h in range(HPC):
                    for r in rs:
                        i = c * 4 + r
                        jl = pv[i]
                        pvp = ps_sm.tile([P, 512], FP32, tag="sm",
                                         name=f"pvp{i}_{h}")
                        for nj, j in enumerate(jl):
                            nc.tensor.matmul(
                                pvp[:, 0:129],
                                pt_of[(i, j)][:, h * 128:(h + 1) * 128],
                                v_sb[:, j * 130:j * 130 + 129],
                                start=(nj == 0), stop=(nj == len(jl) - 1))
                        rd = stage.tile([P, 1], FP32, tag="rd", bufs=4,
                                        name=f"rd{i}_{h}")
                        nc.vector.reciprocal(rd[:], pvp[:, 128:129])
                        nc.vector.scalar_tensor_tensor(
                            agc[:, r * 512 + h * 128: r * 512 + (h + 1) * 128],
                            pvp[:, 0:128], rd[:],
                            sg[:, i * 512 + h * 128: i * 512 + (h + 1) * 128],
                            ALU.mult, ALU.mult)

            def emit_pv_oproj(c, pt_of, agcs_c, rs):
                rs = list(rs)
                emit_pv_houter(c, pt_of, agcs_c, rs)
                for r in rs:
                    _drive(gen_oproj_i(c, r, agcs_c))

            # ================= emission schedule =================
            # PE order per section c:
            #   [proj c+1][PV c][nrt c+1][agT+oproj c][logits c+1]
            # ropes for c+1 drain on vector during PV/oproj of c; logits
            # matmuls then see kT/qT already evicted. exps of c+1 run on
            # scalar under the next section's proj. Chunk 3's first half of
            # PV/oproj is pulled into section 2 to shorten the tail.
            pt_of = {}
            emit_proj_group([0, 1, 2, 3], paired=True)
            emit_nrt_chunk(0)
            emit_logits_chunk(0, pt_of)
            agcs = {}
            for c in range(4):
                if c < 3:
                    emit_proj_group(list(range(4 * (c + 1), 4 * (c + 2))))
                agcs[c] = stage.tile([P, 4 * 512], BF16, tag="agc", bufs=2, name=f"agc{c}")
                nxt = {}
                if c < 3:
                    emit_pv_houter(c, pt_of, agcs[c], range(4))
                    emit_nrt_chunk(c + 1)
                    for r in range(4):
                        _drive(gen_oproj_i(c, r, agcs[c]))
                    emit_logits_chunk(c + 1, nxt)
                    if c == 2:
                        agcs[3] = stage.tile([P, 4 * 512], BF16, tag="agc", bufs=2, name="agc3")
                        emit_pv_oproj(3, nxt, agcs[3], range(2))
                else:
                    emit_pv_oproj(c, pt_of, agcs[c], range(2, 4))
                pt_of = nxt

    split_multiwaits(nc)
    return nc


def _pack(a, ncol):
    """[R*128, ncol-per-row...] -> SBUF layout [128, R*ncol]"""
    R = a.shape[0] // P
    return np.ascontiguousarray(
        a.reshape(R, P, ncol).transpose(1, 0, 2).reshape(P, R * ncol))


def _install_ntff_hook():
    """Best-effort NTFF profiling hook (axon containers); harmless if absent."""
    import contextlib, ctypes, types
    if "antenv.axon_hooks" in sys.modules:
        return
    lib = ctypes.CDLL("/opt/axon/libaxon_pjrt.so")
    if not hasattr(lib, "axon_start_nrt_profile"):
        raise RuntimeError("no profile symbols")
    lib.axon_start_nrt_profile.argtypes = [ctypes.POINTER(ctypes.c_int64), ctypes.c_size_t]
    lib.axon_start_nrt_profile.restype = ctypes.c_int64
    lib.axon_stop_nrt_profile.argtypes = [ctypes.c_char_p]
    lib.axon_stop_nrt_profile.restype = ctypes.c_int64

    @contextlib.contextmanager
    def _hook(output_dir, device_ids):
        import jax
        jax.devices()
        if device_ids:
            ids = (ctypes.c_int64 * len(device_ids))(*device_ids)
            rc = lib.axon_start_nrt_profile(ids, len(device_ids))
        else:
            rc = lib.axon_start_nrt_profile(None, 0)
        if rc != 0:
            raise RuntimeError(f"axon_start_nrt_profile rc={rc}")
        try:
            yield
        finally:
            lib.axon_stop_nrt_profile(str(output_dir).encode())

    store = {"h": _hook}
    mod = types.ModuleType("antenv.axon_hooks")
    mod.get_axon_ntff_profile_hook = lambda: store.get("h")
    mod.set_axon_ntff_profile_hook = lambda h: store.__setitem__("h", h)
    import antenv
    antenv.axon_hooks = mod
    sys.modules["antenv.axon_hooks"] = mod


def kernel(hidden, cos, sin, segment_ids, position_ids, Wq, Wk, Wv, Wo,
           q_norm_w, k_norm_w):
    hidden = np.asarray(hidden, np.float32)
    cos = np.asarray(cos, np.float32)
    sin = np.asarray(sin, np.float32)
    segment_ids = np.asarray(segment_ids)
    position_ids = np.asarray(position_ids)
    Wq = np.asarray(Wq, np.float32)
    Wk = np.asarray(Wk, np.float32)
    Wv = np.asarray(Wv, np.float32)
    Wo = np.asarray(Wo, np.float32)
    q_norm_w = np.asarray(q_norm_w, np.float32)
    k_norm_w = np.asarray(k_norm_w, np.float32)

    cls, mask_idx, masksT, pv = _build_schedule(segment_ids, position_ids)
    n_masks = len(masksT[0])

    rolled_q = np.roll(q_norm_w, -64)
    rolled_k = np.roll(k_norm_w, -64)
    sign = np.where(np.arange(K) < 64, -1.0, 1.0).astype(np.float32)
    in_maps = []
    for core in range(8):
        b, g = core // 4, core % 4
        hidT = np.ascontiguousarray(hidden[b].T)            # [D, T]
        Wq_core = Wq[:, g * 1024:(g + 1) * 1024].reshape(D, HPC, 256)
        Wq_q = np.ascontiguousarray(Wq_core[:, :, :128].reshape(D, 512))
        Wq_g = np.ascontiguousarray(Wq_core[:, :, 128:].reshape(D, 512))
        wkv_core = np.concatenate([Wk[:, g * K:(g + 1) * K],
                                   Wv[:, g * K:(g + 1) * K]], 1)
        if n_masks:
            mk = np.stack(masksT[b], 0)                     # [n, 128s, 128t]
            mk = mk.transpose(1, 0, 2).reshape(P, n_masks * 128)
        else:
            mk = np.zeros((P, 128), np.float32)
        m = dict(
            hid=_pack(hidT, T).astype(BF),
            wq=_pack(Wq_q, 512).astype(BF),
            wkv=_pack(wkv_core, 256).astype(BF),
            wo=_pack(Wo[g * 512:(g + 1) * 512, :], D).astype(BF),
            cq=_pack(cos[b] * q_norm_w[None, :] * SCALE, K).astype(BF),
            sq=_pack(sin[b] * rolled_q[None, :] * sign[None, :] * SCALE, K).astype(BF),
            ck=_pack(cos[b] * k_norm_w[None, :], K).astype(BF),
            sk=_pack(sin[b] * rolled_k[None, :] * sign[None, :], K).astype(BF),
            masks=mk.astype(BF),
        )
        if GATE_F8:
            m["hid8"] = _pack(hidT, T).astype(F8NP)
            # [D,512] -> [pair, sub, p, col] -> [p, pair*sub*col]
            wg8 = (Wq_g * GSC).reshape(8, 2, P, 512).transpose(2, 0, 1, 3)
            m["wg"] = np.ascontiguousarray(wg8.reshape(P, 8 * 1024)).astype(F8NP)
        else:
            m["wg"] = _pack(Wq_g, 512).astype(BF)
        in_maps.append(m)

    nc = _build_program(n_masks, cls, mask_idx, pv)
    res = None
    try:
        _install_ntff_hook()
        res = run_bass_kernel_spmd(nc, in_maps, list(range(8)), trace=True)
    except Exception:
        res = None
    if res is None:
        res = run_bass_kernel_spmd(nc, in_maps, list(range(8)))
    out = np.zeros((B, T, D), np.float32)
    for core in range(8):
        b = core // 4
        out[b] += res.results[core]["out"].astype(np.float32)
    kernel.last_results = res
    return out


if __name__ == "__main__":
    pass
